# revision 14
# baseline (speedup 1.0000x reference)
"""Trainium2 Bass kernel for the shared-weight multi-head attention problem.

Math (per batch b, head h, with x_h = x[b,:,h*64:(h+1)*64] [S, d]):
    q = k = x_h @ W + b
    s = q @ q^T / d               (symmetric!)
    t = s + (1-mask_q) * (-1e6)   (constant per softmax row: cancels in softmax
                                   up to the fp32 quantization of s for masked
                                   rows, which this kernel does NOT reproduce;
                                   measured end-to-end rel err ~1.2e-2 < 2e-2)
    out_h = softmax(t) @ x_h

Device strategy (8 cores, data parallel over (batch, head-group-of-8)):
  - s is symmetric, so only the upper block-triangle of scores (136 of 256
    [128,128] tiles per head) is computed and exp'd; the lower tiles are
    produced by DMA x-bar transposes of the exp'd bf16 tiles (one batched
    3D-AP transpose per block row - no PE/ACT/DVE cost). This halves the
    ACT (exp) work, which is otherwise the bottleneck, and halves the
    scores matmul work on PE.
  - All per-head operands are derived on-chip from one upfront DMA (the
    bf16 [x|1] PV operand): per-head DMA loads into rotating buffers were
    observed to race on HW (fine in CoreSim), so they are avoided.
  - qT is duplicated into both partition halves of a [128,S] tile (dual
    projection matmuls, one writing PSUM at base partition 64) so that two
    64-deep score rows stream through disjoint PE row-group sub-arrays
    concurrently (tile-position row tiling, ~2x score throughput).
  - F tiles live in one [128, 17*S] bf16 arena per head (16 row strips +
    padding for the batched mirror AP); PV consumes strip columns with full
    128-deep contraction; L (softmax denominators) come free from a
    ones-column interleaved into the PV stationary operand.
  - No max-subtraction (exp args in [-1.1, 2.3]).
  - Output is written as [nh, 65, S] (d rows of unnormalized PV + L);
    normalization, transposition to [S, d] and gathering happen on host.
"""

import numpy as np

B, S, D, H, d = 4, 2048, 1024, 16, 64
NH = 8          # heads per core
NCORES = 8
KT = S // 128   # k blocks per head

_NC_CACHE = {}


def _build_nc(s=S, nh=NH, has_bias=False, reps=1, ps_bufs=2, mirror_mode="batched"):
    import concourse.bacc as bacc
    import concourse.tile as tile
    from concourse import mybir
    from concourse.masks import make_identity

    f32 = mybir.dt.float32
    bf16 = mybir.dt.bfloat16
    Exp = mybir.ActivationFunctionType.Exp

    kt = s // 128
    nj = s // 512

    nc = bacc.Bacc("TRN2", target_bir_lowering=False, debug=False)

    # xb: PV stationary operand for all heads, pre-packed on host:
    #   xb[h, p, t*65+c] = x[t*128+p, h*64+c]  (c<64), xb[h, p, t*65+64] = 1.0
    xb_in = nc.declare_dram_parameter("xb", [128, nh * kt * 65], bf16, isOutput=False)
    w_in = nc.declare_dram_parameter("W", [d, d], f32, isOutput=False)
    if has_bias:
        b_in = nc.declare_dram_parameter("b", [d, 1], f32, isOutput=False)
    out = nc.declare_dram_parameter("out", [nh, d + 1, s], f32, isOutput=True)

    with tile.TileContext(nc) as tc:
        with (
            tc.tile_pool(name="const", bufs=1) as const_pool,
            tc.tile_pool(name="xt", bufs=1) as xt_pool,
            tc.tile_pool(name="qt", bufs=2) as qt_pool,
            tc.tile_pool(name="F", bufs=2) as f_pool,
            tc.tile_pool(name="at", bufs=2) as at_pool,
            tc.tile_pool(name="ps", bufs=ps_bufs, space="PSUM") as ps_pool,
            tc.tile_pool(name="po", bufs=2, space="PSUM") as po_pool,
            tc.tile_pool(name="pm", bufs=1, space="PSUM") as pm_pool,
        ):
            ident = const_pool.tile([128, 128], f32, tag="ident")
            make_identity(nc, ident[:])
            ident_bf = const_pool.tile([128, 128], bf16, tag="identbf")
            nc.vector.tensor_copy(ident_bf[:], ident[:])
            w_raw = const_pool.tile([d, d], f32, tag="wraw")
            nc.sync.dma_start(w_raw[:], w_in[:, :])
            w_sb = const_pool.tile([d, d], bf16, tag="w")
            nc.vector.tensor_copy(w_sb[:], w_raw[:])
            b_sb2 = None
            if has_bias:
                b_sb2 = const_pool.tile([128, 1], f32, tag="b")
                nc.sync.dma_start(b_sb2[0:d, :], b_in[:, :])
                nc.sync.dma_start(b_sb2[d : 2 * d, :], b_in[:, :])
            # one upfront load of every head's PV operand (17 KB/partition)
            xball = const_pool.tile([128, nh * kt * 65], bf16, tag="xball")
            # per-head slices so head 0's compute starts after ~1/8 of the load
            for hh in range(nh):
                nc.sync.dma_start(
                    xball[:, hh * kt * 65 : (hh + 1) * kt * 65],
                    xb_in[:, hh * kt * 65 : (hh + 1) * kt * 65],
                )

            def xh_slice(h, t):
                o = h * kt * 65 + t * 65
                return xball[:, o : o + 65]

            def emit_row_pair_scores(qt, F, a):
                """Rows (a, a+1) score+exp chunks, with the two rows' matmuls
                interleaved in the PE queue: row a contracts over qt2
                partitions 0..63 and row a+1 over 64..127, so adjacent
                matmuls land on disjoint PE sub-arrays and run concurrently
                (row-group tiling)."""
                rows = []
                for half, aa in ((0, a), (1, a + 1)):
                    chunks = []
                    pos = aa * 128
                    while pos < s:
                        clen = min(1024, s - pos)
                        chunks.append((pos, clen))
                        pos += clen
                    rows.append((half, aa, chunks))
                nchunks = max(len(r[2]) for r in rows)
                for ci in range(nchunks):
                    work = []
                    for half, aa, chunks in rows:
                        if ci >= len(chunks):
                            continue
                        pos, clen = chunks[ci]
                        ps = ps_pool.tile([128, 1024], f32, tag="ps")
                        work.append((half, aa, pos, clen, ps))
                    for off in (0, 512):
                        for half, aa, pos, clen, ps in work:
                            if off >= clen:
                                continue
                            l0 = min(512, clen - off)
                            p0 = half * d
                            nc.tensor.matmul(
                                ps[:, off : off + l0],
                                qt[p0 : p0 + d, aa * 128 : (aa + 1) * 128],
                                qt[p0 : p0 + d, pos + off : pos + off + l0],
                                start=True, stop=True,
                            )
                    for half, aa, pos, clen, ps in work:
                        nc.scalar.activation(
                            F[:, aa * s + pos : aa * s + pos + clen],
                            ps[:, 0:clen], Exp, bias=0.0, scale=1.0 / 64.0,
                        )

            def emit_row_mirrors(F, a):
                # mirrors: F[:, j2*s + a*128 : +128] = T(F[:, a*s + j2*128 : +128])
                base = a * s
                nb = kt - 1 - a
                if nb <= 0:
                    return
                if mirror_mode == "batched":
                    src = F[:, base + (a + 1) * 128 : base + s]
                    in3 = src.rearrange("p (b f) -> p b f", f=128)
                    db = (a + 1) * s + a * 128
                    out3 = F[:, db : db + nb * s].rearrange(
                        "p (b r) -> p b r", r=s
                    )[:, :, 0:128]
                    nc.sync.dma_start_transpose(out3, in3)
                else:
                    for j2 in range(a + 1, kt):
                        nc.sync.dma_start_transpose(
                            F[:, j2 * s + a * 128 : j2 * s + (a + 1) * 128],
                            F[:, base + j2 * 128 : base + (j2 + 1) * 128],
                        )

            def pv_open(h, F, g0):
                """Two column-groups (g0, g0+1) accumulated together so each
                PV stationary operand is loaded once per two matmuls."""
                poa = po_pool.tile([d + 1, 512], f32, tag="po")
                pob = po_pool.tile([d + 1, 512], f32, tag="po")
                return [h, F, g0, (poa, pob)]

            def pv_quarter(st, qi):
                """Unit qi in [0,8): matmuls for t in [2qi, 2qi+2), both groups."""
                h, F, g0, pos_ = st
                for t in range(2 * qi, 2 * qi + 2):
                    for u in range(2):
                        g = g0 + u
                        nc.tensor.matmul(
                            pos_[u][:],
                            xh_slice(h, t),
                            F[:, t * s + g * 512 : t * s + (g + 1) * 512],
                            start=(t == 0), stop=(t == kt - 1),
                        )

            def pv_close(st):
                h, F, g0, pos_ = st
                for u in range(2):
                    at = at_pool.tile([d + 1, 512], f32, tag="at")
                    nc.vector.tensor_copy(at[:], pos_[u][:])
                    nc.sync.dma_start(
                        out[h, :, (g0 + u) * 512 : (g0 + u + 1) * 512], at[:]
                    )

            def emit_pv(h, F, g0):
                st = pv_open(h, F, g0)
                for qi in range(8):
                    pv_quarter(st, qi)
                pv_close(st)

            # reps > 1 repeats the whole body (for timing-by-slope only)
            for _rep in range(reps):
                prev = None  # (h, F) with pv pending
                for h in range(nh):
                    # prep(h) interleaved with pv(prev, 0): the PE transposes
                    # and projection chains stall on psum round-trips, so the
                    # previous head's first PV group fills the gaps.
                    st = pv_open(prev[0], prev[1], 0) if prev is not None else None
                    xt_t = xt_pool.tile([d, s], bf16, tag="xt")
                    for c4 in range(kt // 4):
                        pt = pm_pool.tile([d, 512], bf16, tag="pt", bufs=1)
                        for u in range(4):
                            t = c4 * 4 + u
                            nc.tensor.transpose(
                                pt[:, u * 128 : (u + 1) * 128],
                                xh_slice(h, t)[:, 0:64],
                                ident_bf[:, :],
                            )
                        nc.vector.tensor_copy(
                            xt_t[:, c4 * 512 : (c4 + 1) * 512], pt[:]
                        )
                        if st is not None:
                            pv_quarter(st, c4)
                    # qT duplicated into both partition halves of qt2 so two
                    # score rows can row-tile the PE concurrently (64-deep
                    # contractions at tile_position rows 0 and 64).
                    qt = qt_pool.tile([128, s], bf16, tag="qt")
                    for jj in range(nj):
                        pm = pm_pool.tile([128, 512], f32, tag="pm")
                        nc.tensor.matmul(
                            pm[0:d, :], w_sb[:], xt_t[:, jj * 512 : (jj + 1) * 512],
                            start=True, stop=True,
                        )
                        nc.tensor.matmul(
                            pm[d:2 * d, :], w_sb[:], xt_t[:, jj * 512 : (jj + 1) * 512],
                            start=True, stop=True,
                        )
                        if has_bias:
                            nc.vector.tensor_scalar_add(
                                qt[:, jj * 512 : (jj + 1) * 512], pm[:],
                                b_sb2[:],
                            )
                        else:
                            nc.vector.tensor_copy(
                                qt[:, jj * 512 : (jj + 1) * 512], pm[:]
                            )
                        if st is not None:
                            pv_quarter(st, 4 + jj)
                    if st is not None:
                        pv_close(st)
                    # 16 row strips + one strip of padding for the mirror AP
                    F = f_pool.tile([128, (kt + 1) * s], bf16, tag="F")
                    # rows(h, g) interleaved row-by-row with pv(prev, g+1)
                    st = pv_open(prev[0], prev[1], 2) if prev is not None else None
                    for a in range(0, kt, 2):
                        emit_row_pair_scores(qt, F, a)
                        emit_row_mirrors(F, a)
                        emit_row_mirrors(F, a + 1)
                        if st is not None:
                            pv_quarter(st, a // 2)
                    if st is not None:
                        pv_close(st)
                    prev = (h, F)
                for g0 in (0, 2):
                    emit_pv(prev[0], prev[1], g0)

    nc.compile()
    return nc


def get_nc(s=S, nh=NH, has_bias=False, reps=1, ps_bufs=2, mirror_mode="batched"):
    key = (s, nh, has_bias, reps, ps_bufs, mirror_mode)
    if key not in _NC_CACHE:
        _NC_CACHE[key] = _build_nc(s, nh, has_bias, reps, ps_bufs, mirror_mode)
    return _NC_CACHE[key]


def make_in_maps(x, W, b, has_bias, s=S, nh=NH):
    """Shard full inputs into per-core input maps (core = batch*2 + head_group)."""
    import ml_dtypes

    x = np.asarray(x, dtype=np.float32)
    W = np.ascontiguousarray(np.asarray(W, dtype=np.float32))
    bv = np.ascontiguousarray(np.asarray(b, dtype=np.float32).reshape(d, 1))
    kt = s // 128
    in_maps = []
    for c in range(NCORES):
        bb, hg = c // 2, c % 2
        xs = x[bb][:, hg * nh * d : (hg + 1) * nh * d]  # [s, nh*d] f32
        # xb[p, h*kt*65 + t*65 + c] = xs[t*128+p, h*64+c]; 65th col = 1.0
        xb = np.ones((128, nh, kt, 65), dtype=np.float32)
        x4 = xs.reshape(kt, 128, nh, d)  # [t, p, h, c]
        xb[:, :, :, :64] = x4.transpose(1, 2, 0, 3)
        xb = xb.reshape(128, nh * kt * 65)
        m = {
            "xb": xb.astype(ml_dtypes.bfloat16),
            "W": W,
        }
        if has_bias:
            m["b"] = bv
        in_maps.append(m)
    return in_maps


def gather_out(results):
    """results: list of 8 dicts with 'out' [NH, d+1, S] -> full [B, S, D]."""
    a = np.empty((B, H, S, d), np.float32)
    for c in range(NCORES):
        bb, hg = c // 2, c % 2
        o = np.asarray(results[c]["out"])  # [nh, 65, s]
        num = o[:, :d, :]
        den = o[:, d, :]
        a[bb, hg * NH : (hg + 1) * NH] = np.transpose(
            num / den[:, None, :], (0, 2, 1)
        )
    return a.reshape(B, S, D)


def kernel(x, mask, W, b):
    from concourse.bass_utils import run_bass_kernel_spmd

    has_bias = bool(np.any(np.asarray(b)))
    nc = get_nc(has_bias=has_bias)
    in_maps = make_in_maps(x, W, b, has_bias)
    res = run_bass_kernel_spmd(nc, in_maps, list(range(NCORES)))
    return gather_out(res.results)


# revision 15
# speedup vs baseline: 1.1898x; 1.1898x over previous
"""Trainium2 Bass kernel for the shared-weight multi-head attention problem.

Math (per batch b, head h, with x_h = x[b,:,h*64:(h+1)*64] [S, d]):
    q = k = x_h @ W + b
    s = q @ q^T / d               (symmetric!)
    t = s + (1-mask_q) * (-1e6)   (constant per softmax row: cancels in softmax
                                   up to the fp32 quantization of s for masked
                                   rows, which this kernel does NOT reproduce;
                                   measured end-to-end rel err ~1.2e-2 < 2e-2)
    out_h = softmax(t) @ x_h

Device strategy (8 cores, data parallel over (batch, head-group-of-8)):
  - s is symmetric, so only the upper block-triangle of scores (136 of 256
    [128,128] tiles per head) is computed and exp'd; the lower tiles are
    produced by DMA x-bar transposes of the exp'd bf16 tiles (one batched
    3D-AP transpose per block row - no PE/ACT/DVE cost). This halves the
    ACT (exp) work, which is otherwise the bottleneck, and halves the
    scores matmul work on PE.
  - All per-head operands are derived on-chip from one upfront DMA (the
    bf16 [x|1] PV operand): per-head DMA loads into rotating buffers were
    observed to race on HW (fine in CoreSim), so they are avoided.
  - qT is duplicated into both partition halves of a [128,S] tile (dual
    projection matmuls, one writing PSUM at base partition 64) so that two
    64-deep score rows stream through disjoint PE row-group sub-arrays
    concurrently (tile-position row tiling, ~2x score throughput).
  - F tiles live in one [128, 17*S] bf16 arena per head (16 row strips +
    padding for the batched mirror AP); PV consumes strip columns with full
    128-deep contraction; L (softmax denominators) come free from a
    ones-column interleaved into the PV stationary operand.
  - No max-subtraction (exp args in [-1.1, 2.3]).
  - Output is written as [nh, 65, S] (d rows of unnormalized PV + L);
    normalization, transposition to [S, d] and gathering happen on host.
"""

import numpy as np

B, S, D, H, d = 4, 2048, 1024, 16, 64
NH = 8          # heads per core
NCORES = 8
KT = S // 128   # k blocks per head

_NC_CACHE = {}


def _build_nc(s=S, nh=NH, has_bias=False, reps=1, ps_bufs=2, mirror_mode="batched"):
    import concourse.bacc as bacc
    import concourse.tile as tile
    from concourse import mybir
    from concourse.masks import make_identity

    f32 = mybir.dt.float32
    bf16 = mybir.dt.bfloat16
    Exp = mybir.ActivationFunctionType.Exp

    kt = s // 128
    nj = s // 512

    nc = bacc.Bacc("TRN2", target_bir_lowering=False, debug=False)

    # xb: PV stationary operand for all heads, pre-packed on host:
    #   xb[h, p, t*65+c] = x[t*128+p, h*64+c]  (c<64), xb[h, p, t*65+64] = 1.0
    xb_in = nc.declare_dram_parameter("xb", [128, nh * kt * 65], bf16, isOutput=False)
    w_in = nc.declare_dram_parameter("W", [d, d], f32, isOutput=False)
    if has_bias:
        b_in = nc.declare_dram_parameter("b", [d, 1], f32, isOutput=False)
    out = nc.declare_dram_parameter("out", [nh, d + 1, s], f32, isOutput=True)

    with tile.TileContext(nc) as tc:
        with (
            tc.tile_pool(name="const", bufs=1) as const_pool,
            tc.tile_pool(name="xt", bufs=1) as xt_pool,
            tc.tile_pool(name="qt", bufs=2) as qt_pool,
            tc.tile_pool(name="F", bufs=2) as f_pool,
            tc.tile_pool(name="at", bufs=2) as at_pool,
            tc.tile_pool(name="ps", bufs=ps_bufs, space="PSUM") as ps_pool,
            tc.tile_pool(name="po", bufs=2, space="PSUM") as po_pool,
            tc.tile_pool(name="pm", bufs=1, space="PSUM") as pm_pool,
        ):
            ident = const_pool.tile([128, 128], f32, tag="ident")
            make_identity(nc, ident[:])
            ident_bf = const_pool.tile([128, 128], bf16, tag="identbf")
            nc.vector.tensor_copy(ident_bf[:], ident[:])
            w_raw = const_pool.tile([d, d], f32, tag="wraw")
            nc.sync.dma_start(w_raw[:], w_in[:, :])
            w_sb = const_pool.tile([d, d], bf16, tag="w")
            nc.vector.tensor_copy(w_sb[:], w_raw[:])
            b_sb2 = None
            if has_bias:
                b_sb2 = const_pool.tile([128, 1], f32, tag="b")
                nc.sync.dma_start(b_sb2[0:d, :], b_in[:, :])
                nc.sync.dma_start(b_sb2[d : 2 * d, :], b_in[:, :])
            # one upfront load of every head's PV operand (17 KB/partition)
            xball = const_pool.tile([128, nh * kt * 65], bf16, tag="xball")
            # per-head slices so head 0's compute starts after ~1/8 of the load
            for hh in range(nh):
                nc.sync.dma_start(
                    xball[:, hh * kt * 65 : (hh + 1) * kt * 65],
                    xb_in[:, hh * kt * 65 : (hh + 1) * kt * 65],
                )

            def xh_slice(h, t):
                o = h * kt * 65 + t * 65
                return xball[:, o : o + 65]

            def emit_row_pair_scores(qt, F, a):
                """Rows (a, a+1) score+exp chunks, with the two rows' matmuls
                interleaved in the PE queue: row a contracts over qt2
                partitions 0..63 and row a+1 over 64..127, so adjacent
                matmuls land on disjoint PE sub-arrays and run concurrently
                (row-group tiling)."""
                rows = []
                for half, aa in ((0, a), (1, a + 1)):
                    chunks = []
                    pos = aa * 128
                    while pos < s:
                        clen = min(1024, s - pos)
                        chunks.append((pos, clen))
                        pos += clen
                    rows.append((half, aa, chunks))
                nchunks = max(len(r[2]) for r in rows)
                for ci in range(nchunks):
                    work = []
                    for half, aa, chunks in rows:
                        if ci >= len(chunks):
                            continue
                        pos, clen = chunks[ci]
                        ps = ps_pool.tile([128, 1024], f32, tag="ps")
                        work.append((half, aa, pos, clen, ps))
                    for off in (0, 512):
                        for half, aa, pos, clen, ps in work:
                            if off >= clen:
                                continue
                            l0 = min(512, clen - off)
                            p0 = half * d
                            nc.tensor.matmul(
                                ps[:, off : off + l0],
                                qt[p0 : p0 + d, aa * 128 : (aa + 1) * 128],
                                qt[p0 : p0 + d, pos + off : pos + off + l0],
                                start=True, stop=True,
                            )
                    for half, aa, pos, clen, ps in work:
                        nc.scalar.activation(
                            F[:, aa * s + pos : aa * s + pos + clen],
                            ps[:, 0:clen], Exp, bias=0.0, scale=1.0 / 64.0,
                        )

            def emit_row_mirrors(F, a):
                # mirrors: F[:, j2*s + a*128 : +128] = T(F[:, a*s + j2*128 : +128])
                base = a * s
                nb = kt - 1 - a
                if nb <= 0:
                    return
                if mirror_mode == "batched":
                    src = F[:, base + (a + 1) * 128 : base + s]
                    in3 = src.rearrange("p (b f) -> p b f", f=128)
                    db = (a + 1) * s + a * 128
                    out3 = F[:, db : db + nb * s].rearrange(
                        "p (b r) -> p b r", r=s
                    )[:, :, 0:128]
                    nc.sync.dma_start_transpose(out3, in3)
                else:
                    for j2 in range(a + 1, kt):
                        nc.sync.dma_start_transpose(
                            F[:, j2 * s + a * 128 : j2 * s + (a + 1) * 128],
                            F[:, base + j2 * 128 : base + (j2 + 1) * 128],
                        )

            def pv_open(h, F, g0):
                """Two column-groups (g0, g0+1) accumulated together so each
                PV stationary operand is loaded once per two matmuls."""
                poa = po_pool.tile([d + 1, 512], f32, tag="po")
                pob = po_pool.tile([d + 1, 512], f32, tag="po")
                return [h, F, g0, (poa, pob)]

            def pv_quarter(st, qi):
                """Unit qi in [0,8): matmuls for t in [2qi, 2qi+2), both groups."""
                h, F, g0, pos_ = st
                for t in range(2 * qi, 2 * qi + 2):
                    for u in range(2):
                        g = g0 + u
                        nc.tensor.matmul(
                            pos_[u][:],
                            xh_slice(h, t),
                            F[:, t * s + g * 512 : t * s + (g + 1) * 512],
                            start=(t == 0), stop=(t == kt - 1),
                        )

            def pv_close(st):
                h, F, g0, pos_ = st
                for u in range(2):
                    at = at_pool.tile([d + 1, 512], f32, tag="at")
                    nc.vector.tensor_copy(at[:], pos_[u][:])
                    nc.sync.dma_start(
                        out[h, :, (g0 + u) * 512 : (g0 + u + 1) * 512], at[:]
                    )

            def emit_pv(h, F, g0):
                st = pv_open(h, F, g0)
                for qi in range(8):
                    pv_quarter(st, qi)
                pv_close(st)

            # reps > 1 repeats the whole body (for timing-by-slope only)
            for _rep in range(reps):
                prev = None  # (h, F) with pv pending
                for h in range(nh):
                    # prep(h) interleaved with pv(prev, 0): the PE transposes
                    # and projection chains stall on psum round-trips, so the
                    # previous head's first PV group fills the gaps.
                    st = pv_open(prev[0], prev[1], 0) if prev is not None else None
                    xt_t = xt_pool.tile([d, s], bf16, tag="xt")
                    for c4 in range(kt // 4):
                        pt = pm_pool.tile([d, 512], bf16, tag="pt", bufs=1)
                        for u in range(4):
                            t = c4 * 4 + u
                            nc.tensor.transpose(
                                pt[:, u * 128 : (u + 1) * 128],
                                xh_slice(h, t)[:, 0:64],
                                ident_bf[:, :],
                            )
                        nc.vector.tensor_copy(
                            xt_t[:, c4 * 512 : (c4 + 1) * 512], pt[:]
                        )
                        if st is not None:
                            pv_quarter(st, c4)
                    # qT duplicated into both partition halves of qt2 so two
                    # score rows can row-tile the PE concurrently (64-deep
                    # contractions at tile_position rows 0 and 64).
                    qt = qt_pool.tile([128, s], bf16, tag="qt")
                    for jj in range(nj):
                        pm = pm_pool.tile([128, 512], f32, tag="pm")
                        nc.tensor.matmul(
                            pm[0:d, :], w_sb[:], xt_t[:, jj * 512 : (jj + 1) * 512],
                            start=True, stop=True,
                        )
                        nc.tensor.matmul(
                            pm[d:2 * d, :], w_sb[:], xt_t[:, jj * 512 : (jj + 1) * 512],
                            start=True, stop=True,
                        )
                        if has_bias:
                            nc.vector.tensor_scalar_add(
                                qt[:, jj * 512 : (jj + 1) * 512], pm[:],
                                b_sb2[:],
                            )
                        else:
                            nc.vector.tensor_copy(
                                qt[:, jj * 512 : (jj + 1) * 512], pm[:]
                            )
                        if st is not None:
                            pv_quarter(st, 4 + jj)
                    if st is not None:
                        pv_close(st)
                    # 16 row strips + one strip of padding for the mirror AP
                    F = f_pool.tile([128, (kt + 1) * s], bf16, tag="F")
                    # rows(h, g) interleaved row-by-row with pv(prev, g+1)
                    st = pv_open(prev[0], prev[1], 2) if prev is not None else None
                    st_own = None
                    last = h == nh - 1
                    for p, a in enumerate(range(0, kt, 2)):
                        emit_row_pair_scores(qt, F, a)
                        emit_row_mirrors(F, a)
                        emit_row_mirrors(F, a + 1)
                        if not last:
                            if st is not None:
                                pv_quarter(st, p)
                        else:
                            # last head: drain pv(prev) at double rate over the
                            # first four pairs, then interleave this head's own
                            # first two pv groups (rows 0..7 are exp'd by then)
                            if st is not None and p < 4:
                                pv_quarter(st, 2 * p)
                                pv_quarter(st, 2 * p + 1)
                            if p == 3:
                                if st is not None:
                                    pv_close(st)
                                    st = None
                                st_own = pv_open(h, F, 0)
                            if st_own is not None and p >= 4:
                                pv_quarter(st_own, 2 * (p - 4))
                                pv_quarter(st_own, 2 * (p - 4) + 1)
                    if st is not None:
                        pv_close(st)
                    if st_own is not None:
                        pv_close(st_own)
                    prev = (h, F)
                # epilogue: only the last head's upper column half remains
                emit_pv(prev[0], prev[1], 2)

    nc.compile()
    return nc


def get_nc(s=S, nh=NH, has_bias=False, reps=1, ps_bufs=2, mirror_mode="batched"):
    key = (s, nh, has_bias, reps, ps_bufs, mirror_mode)
    if key not in _NC_CACHE:
        _NC_CACHE[key] = _build_nc(s, nh, has_bias, reps, ps_bufs, mirror_mode)
    return _NC_CACHE[key]


def make_in_maps(x, W, b, has_bias, s=S, nh=NH):
    """Shard full inputs into per-core input maps (core = batch*2 + head_group)."""
    import ml_dtypes

    x = np.asarray(x, dtype=np.float32)
    W = np.ascontiguousarray(np.asarray(W, dtype=np.float32))
    bv = np.ascontiguousarray(np.asarray(b, dtype=np.float32).reshape(d, 1))
    kt = s // 128
    in_maps = []
    for c in range(NCORES):
        bb, hg = c // 2, c % 2
        xs = x[bb][:, hg * nh * d : (hg + 1) * nh * d]  # [s, nh*d] f32
        # xb[p, h*kt*65 + t*65 + c] = xs[t*128+p, h*64+c]; 65th col = 1.0
        xb = np.ones((128, nh, kt, 65), dtype=np.float32)
        x4 = xs.reshape(kt, 128, nh, d)  # [t, p, h, c]
        xb[:, :, :, :64] = x4.transpose(1, 2, 0, 3)
        xb = xb.reshape(128, nh * kt * 65)
        m = {
            "xb": xb.astype(ml_dtypes.bfloat16),
            "W": W,
        }
        if has_bias:
            m["b"] = bv
        in_maps.append(m)
    return in_maps


def gather_out(results):
    """results: list of 8 dicts with 'out' [NH, d+1, S] -> full [B, S, D]."""
    a = np.empty((B, H, S, d), np.float32)
    for c in range(NCORES):
        bb, hg = c // 2, c % 2
        o = np.asarray(results[c]["out"])  # [nh, 65, s]
        num = o[:, :d, :]
        den = o[:, d, :]
        a[bb, hg * NH : (hg + 1) * NH] = np.transpose(
            num / den[:, None, :], (0, 2, 1)
        )
    return a.reshape(B, S, D)


def kernel(x, mask, W, b):
    from concourse.bass_utils import run_bass_kernel_spmd

    has_bias = bool(np.any(np.asarray(b)))
    nc = get_nc(has_bias=has_bias)
    in_maps = make_in_maps(x, W, b, has_bias)
    res = run_bass_kernel_spmd(nc, in_maps, list(range(NCORES)))
    return gather_out(res.results)
